# revision 13
# baseline (speedup 1.0000x reference)
"""Trainium2 Bass kernel for nn_Attention_21079699489228.

Bahdanau-style attention:
    att1 = enc @ W_enc + b_enc                     [B, N, A]
    att2 = dec @ W_dec + b_dec                     [B, A]
    s    = relu(att1 * att2[:,None,:]) @ w_full + b_full    [B, N]
    att  = s * mask ; alpha = softmax(att, axis=1) * mask
    awe  = einsum('bne,bn->be', enc, alpha)
Returns (awe, alpha).

Sharding: data-parallel over batch B=32 across 8 cores (4 batches/core),
params replicated.  Each core reads its encoder_out shard once (32 MiB),
which is the memory roofline (~94 us/core at 358 GB/s).

Per core, per batch, N=4096 processed in 8 chunks of 512:
  - DMA enc chunk natural [128p(n), 4j, 512e]    (n = (4c+j)*128 + p)
  - (bf16 mode) GpSimd casts the chunk to bf16 for the score path
  - PE 128x128 block transposes -> enc_T[q] [128p(e), 512(n)] per e-chunk q
  - PE att1_T: u[A=128p, 512n] = sum_q W_enc[q].T @ enc_T[q]
  - ACT: v = relu(att2_b * u + att2_b*b_enc)  (per-partition scale/bias)
  - PE score: S_T2[:, 2col:2col+2] = v[:, j-block].T @ [w|w]  -> scores with
    n on partitions (2 moving columns because non-fp32 matmul with a single
    moving column is illegal ISA; odd columns are a discarded duplicate)
  - softmax over the batch's scores [128, 32] without max subtraction
    (scores are O(1); exp never overflows); the denominator includes the
    exp(0)=1 terms of masked positions exactly as the reference does
  - PE awe: accumulate alpha-col.T @ enc-nat-block into [1, 512]

Precision modes (ATT_PREC): "bf16" (score path bf16, fast), "f32r"
(single-pass fp32-high matmuls, ~2e-4), "f32" (exact, 4x slower PE).
ATT_AWE selects the weighted-sum operand dtype ("f32r" default, "bf16").
The alpha output always flows through the f32r/f32 path.
"""

import os
import sys

import numpy as np

B, N, E, A = 32, 4096, 512, 128
NCORES = 8
BPC = B // NCORES            # batches per core
CHUNK = 512                  # n per chunk
NCHUNK = N // CHUNK          # 8
NBLK = CHUNK // 128          # 4  (128-blocks per chunk)
NCOL = N // 128              # 32 (score columns per batch)
NQ = E // 128                # 4  (e-chunks)

PREC = os.environ.get("ATT_PREC", "bf16")    # bf16 | f32r | f32
AWE = os.environ.get("ATT_AWE", "f32r")      # f32r | bf16


def _ensure_path():
    for p in ("/opt/trn_rl_repo", "/root/.axon_site/_ro/trn_rl_repo"):
        if os.path.isdir(p) and p not in sys.path:
            sys.path.append(p)


_NC_CACHE = {}
LAST_RESULT = None


def build_nc(prec=PREC, awe_dt_name=AWE):
    """Build the (single-program, run-on-8-cores) Bass module."""
    _ensure_path()
    from contextlib import ExitStack

    import concourse.bacc as bacc
    import concourse.mybir as mybir
    import concourse.tile as tile
    from concourse.masks import make_identity

    f32 = mybir.dt.float32
    bf16 = mybir.dt.bfloat16
    f32r = mybir.dt.float32r
    # score-path matmul dtype and awe matmul dtype
    sdt = {"bf16": bf16, "f32r": f32r, "f32": f32}[prec]
    adt = {"bf16": bf16, "f32r": f32r, "f32": f32}[
        awe_dt_name if prec == "bf16" else ("f32" if prec == "f32" else "f32r")]
    Act = mybir.ActivationFunctionType
    Op = mybir.AluOpType

    def rcast(ap, dt):
        # reinterpret an fp32 AP as float32r (same 4-byte layout)
        return ap.bitcast(dt) if dt == f32r else ap

    nc = bacc.Bacc("TRN2", debug=False)

    enc_d = nc.declare_dram_parameter("enc", [BPC, N, E], f32, isOutput=False)
    mask_d = nc.declare_dram_parameter("mask", [BPC, N], f32, isOutput=False)
    dh_d = nc.declare_dram_parameter("dh", [BPC, E], f32, isOutput=False)
    wenc_d = nc.declare_dram_parameter("wenc", [E, A], f32, isOutput=False)
    benc_d = nc.declare_dram_parameter("benc", [A], f32, isOutput=False)
    wdec_d = nc.declare_dram_parameter("wdec", [E, A], f32, isOutput=False)
    bdec_d = nc.declare_dram_parameter("bdec", [A], f32, isOutput=False)
    wful_d = nc.declare_dram_parameter("wful", [A], f32, isOutput=False)
    bful_d = nc.declare_dram_parameter("bful", [1], f32, isOutput=False)
    awe_o = nc.declare_dram_parameter("awe_out", [BPC, E], f32, isOutput=True)
    alpha_o = nc.declare_dram_parameter("alpha_out", [BPC, N], f32, isOutput=True)

    with tile.TileContext(nc) as tc, ExitStack() as ctx:
        const = ctx.enter_context(tc.tile_pool(name="const", bufs=1))
        nat_bufs = 4 if (prec == "bf16" and adt == bf16) else 17
        nat_pool = ctx.enter_context(tc.tile_pool(name="nat", bufs=nat_bufs))
        if prec == "bf16":
            natbf_bufs = 17 if adt == bf16 else 3
            natbf_pool = ctx.enter_context(
                tc.tile_pool(name="natbf", bufs=natbf_bufs))
        encT_pool = ctx.enter_context(tc.tile_pool(name="encT", bufs=8))
        v_pool = ctx.enter_context(tc.tile_pool(name="v", bufs=3))
        small = ctx.enter_context(tc.tile_pool(name="small", bufs=3))
        outb = ctx.enter_context(tc.tile_pool(name="outb", bufs=2))
        tp_psum = ctx.enter_context(
            tc.tile_pool(name="tp_psum", bufs=3, space="PSUM"))
        u_psum = ctx.enter_context(
            tc.tile_pool(name="u_psum", bufs=1, space="PSUM"))
        sm_psum = ctx.enter_context(
            tc.tile_pool(name="sm_psum", bufs=2, space="PSUM"))
        awe_psum = ctx.enter_context(
            tc.tile_pool(name="awe_psum", bufs=2, space="PSUM"))

        # ---- constants ----
        identity = const.tile([128, 128], f32)
        make_identity(nc, identity[:])
        if sdt != f32:
            identity_s = const.tile([128, 128], sdt)
            nc.vector.tensor_copy(out=identity_s[:], in_=identity[:])
        else:
            identity_s = identity
        if sdt != f32:
            # f32r identity for the alpha-output transpose (must come from a
            # compute producer so the verifier sees an f32r-rounded buffer)
            identity_r = const.tile([128, 128], f32r)
            nc.vector.tensor_copy(out=identity_r[:], in_=identity[:])
        else:
            identity_r = identity

        # W_enc in [e_part, q, a] layout, in the score-path dtype
        wenc_t = const.tile([128, NQ, A], sdt)
        if sdt == bf16:
            wenc_f = const.tile([128, NQ, A], f32)
            nc.sync.dma_start(out=wenc_f[:], in_=wenc_d[:].rearrange(
                "(q p) a -> p q a", p=128))
            nc.vector.tensor_copy(out=wenc_t[:], in_=wenc_f[:])
        else:
            nc.sync.dma_start(out=wenc_t[:], in_=rcast(wenc_d[:].rearrange(
                "(q p) a -> p q a", p=128), sdt))
        wdt = f32r if sdt != f32 else f32
        wdec_t = const.tile([128, NQ, A], wdt)
        nc.sync.dma_start(out=wdec_t[:], in_=rcast(wdec_d[:].rearrange(
            "(q p) a -> p q a", p=128), wdt))
        benc_c = const.tile([128, 1], f32)
        nc.sync.dma_start(out=benc_c[:], in_=benc_d[:].rearrange(
            "(p o) -> p o", o=1))
        bdec_c = const.tile([128, 1], f32)
        nc.sync.dma_start(out=bdec_c[:], in_=bdec_d[:].rearrange(
            "(p o) -> p o", o=1))
        wful_c = const.tile([128, 1], f32)
        nc.sync.dma_start(out=wful_c[:], in_=wful_d[:].rearrange(
            "(p o) -> p o", o=1))
        # two identical columns of w_full in the score dtype (the score
        # matmul needs >= 2 moving columns for non-fp32 dtypes)
        wful2 = const.tile([128, 2], sdt)
        nc.vector.tensor_copy(out=wful2[:, 0:1], in_=wful_c[:])
        nc.vector.tensor_copy(out=wful2[:, 1:2], in_=wful_c[:])
        bful_c = const.tile([128, 1], f32)
        nc.gpsimd.dma_start(out=bful_c[:], in_=bful_d[:].to_broadcast([128, 1]))
        ones_col = const.tile([128, 1], f32)
        nc.vector.memset(ones_col[:], 1.0)
        ones_row = const.tile([1, 128], f32)
        nc.vector.memset(ones_row[:], 1.0)

        # ---- att2_T = (W_dec.T @ dh.T + b_dec) : [A=128, BPC] ----
        dh_t = const.tile([BPC, E], f32)
        nc.sync.dma_start(out=dh_t[:], in_=dh_d[:])
        dhT = const.tile([128, NQ, BPC], wdt)
        for q in range(NQ):
            pt = sm_psum.tile([128, BPC], f32, tag="sm")
            nc.tensor.matmul(pt[:], dh_t[:, q * 128:(q + 1) * 128],
                             identity[0:BPC, 0:BPC], is_transpose=True)
            nc.vector.tensor_copy(out=dhT[:, q], in_=pt[:])
        u2 = sm_psum.tile([A, BPC], f32, tag="sm")
        for q in range(NQ):
            nc.tensor.matmul(u2[:], wdec_t[:, q], dhT[:, q],
                             start=(q == 0), stop=(q == NQ - 1))
        att2T = const.tile([A, BPC], f32)
        nc.scalar.activation(att2T[:], u2[:], Act.Identity, bias=bdec_c[:])
        # bbb = att2 * b_enc  (per-batch bias for the fused relu)
        bbb = const.tile([A, BPC], f32)
        nc.vector.tensor_scalar(out=bbb[:], in0=att2T[:], scalar1=benc_c[:],
                                scalar2=None, op0=Op.mult)

        # ---- main loop over this core's batches ----
        # Software-pipelined: phase1(b) is emitted before phase2/3(b-1) so
        # the in-order PE queue always has dense transpose/matmul work while
        # the previous batch's softmax serial chain resolves on ACT/DVE.
        state = {}

        def phase1(b):
            # mask[b] -> mask_T [128, NCOL] with n = col*128 + p
            mnat = small.tile([NCOL, 128], f32, tag="mnat")
            nc.sync.dma_start(out=mnat[:], in_=mask_d[b].rearrange(
                "(r c) -> r c", c=128))
            mt_ps = sm_psum.tile([128, NCOL], f32, tag="sm")
            nc.tensor.matmul(mt_ps[:], mnat[:], identity[0:NCOL, 0:NCOL],
                             is_transpose=True)
            maskT = small.tile([128, NCOL], f32, tag="maskT")
            nc.vector.tensor_copy(out=maskT[:], in_=mt_ps[:])

            # scores land in even columns; odd columns are the duplicate
            S_T2 = sm_psum.tile([128, 2 * NCOL], f32, tag="sm")
            awes = []   # per-batch rhs tiles for the awe matmuls
            ndt = f32r if (adt == f32r or sdt == f32r) else f32
            for c in range(NCHUNK):
                nat = nat_pool.tile([128, NBLK, CHUNK], ndt, tag="nat")
                nc.sync.dma_start(
                    out=nat[:],
                    in_=rcast(enc_d[b, c * CHUNK:(c + 1) * CHUNK, :].rearrange(
                        "(j p) e -> p j e", p=128), ndt))
                if prec == "bf16":
                    natbf = natbf_pool.tile([128, NBLK, CHUNK], bf16,
                                            tag="natbf")
                    nc.gpsimd.tensor_copy(out=natbf[:], in_=nat[:].bitcast(f32))
                    tsrc = natbf
                else:
                    tsrc = nat
                awes.append(tsrc if adt == bf16 else nat)

                encT = []
                for q in range(NQ):
                    tpt = tp_psum.tile([128, CHUNK], sdt, tag="tp")
                    for j in range(NBLK):
                        nc.tensor.matmul(
                            tpt[:, j * 128:(j + 1) * 128],
                            rcast(tsrc[:, j, q * 128:(q + 1) * 128], sdt),
                            identity_s[:],
                            is_transpose=True,
                            start=(j == 0), stop=(j == NBLK - 1))
                    et = encT_pool.tile([128, CHUNK], sdt, tag="encT")
                    if q % 2 == 0:
                        nc.vector.tensor_copy(out=et[:], in_=tpt[:])
                    else:
                        nc.scalar.copy(out=et[:], in_=tpt[:])
                    encT.append(et)

                u = u_psum.tile([A, CHUNK], f32, tag="u")
                for q in range(NQ):
                    nc.tensor.matmul(u[:], wenc_t[:, q], encT[q][:],
                                     start=(q == 0), stop=(q == NQ - 1))

                v = v_pool.tile([A, CHUNK], sdt, tag="v")
                nc.scalar.activation(v[:], u[:], Act.Relu,
                                     bias=bbb[:, b:b + 1],
                                     scale=att2T[:, b:b + 1])

                for j in range(NBLK):
                    col = c * NBLK + j
                    nc.tensor.matmul(S_T2[:, 2 * col:2 * col + 2],
                                     v[:, j * 128:(j + 1) * 128],
                                     wful2[:],
                                     start=(col == 0), stop=(col == NCOL - 1))
            state[b] = (maskT, S_T2, awes)

        def phase23(b):
            maskT, S_T2, awes = state.pop(b)
            S_T = S_T2[:].rearrange("p (c two) -> p c two", two=2)[:, :, 0]
            # ---- softmax over this batch ----
            t_mask = small.tile([128, NCOL], f32, tag="tmask")
            nc.vector.scalar_tensor_tensor(out=t_mask[:], in0=S_T,
                                           scalar=bful_c[:], in1=maskT[:],
                                           op0=Op.add, op1=Op.mult)
            p_exp = small.tile([128, NCOL], f32, tag="pexp")
            psum_col = small.tile([128, 1], f32, tag="pcol")
            nc.scalar.activation(p_exp[:], t_mask[:], Act.Exp,
                                 accum_out=psum_col[:])
            d_ps = sm_psum.tile([1, 1], f32, tag="sm")
            nc.tensor.matmul(d_ps[:], ones_col[:], psum_col[:])
            d_sb = small.tile([1, 1], f32, tag="dsb")
            nc.vector.tensor_copy(out=d_sb[:], in_=d_ps[:])
            r_sb = small.tile([1, 1], f32, tag="rsb")
            nc.vector.reciprocal(r_sb[:], d_sb[:])
            rb_ps = sm_psum.tile([128, 1], f32, tag="sm")
            nc.tensor.matmul(rb_ps[:], ones_row[:], r_sb[:])
            rb = small.tile([128, 1], f32, tag="rb")
            nc.vector.tensor_copy(out=rb[:], in_=rb_ps[:])
            # alpha = (exp * r) * mask   (f32r path feeds the alpha output)
            aldt = f32r if sdt != f32 else f32
            alpha = small.tile([128, NCOL], aldt, tag="alpha")
            nc.vector.scalar_tensor_tensor(out=alpha[:], in0=p_exp[:],
                                           scalar=rb[:], in1=maskT[:],
                                           op0=Op.mult, op1=Op.mult)
            if adt != aldt:
                alpha_a = small.tile([128, NCOL], adt, tag="alphaa")
                nc.vector.tensor_copy(out=alpha_a[:], in_=alpha[:])
            else:
                alpha_a = alpha

            # ---- awe = sum_n alpha_n * enc_n ----
            awe_ps = awe_psum.tile([1, E], f32, tag="awe")
            for c in range(NCHUNK):
                for j in range(NBLK):
                    col = c * NBLK + j
                    nc.tensor.matmul(awe_ps[:], alpha_a[:, col:col + 1],
                                     rcast(awes[c][:, j, :], adt),
                                     start=(col == 0), stop=(col == NCOL - 1))
            del awes
            awe_sb = outb.tile([1, E], f32, tag="awesb")
            nc.vector.tensor_copy(out=awe_sb[:], in_=awe_ps[:])
            nc.sync.dma_start(out=awe_o[b].rearrange("(o e) -> o e", o=1),
                              in_=awe_sb[:])

            # ---- alpha out: [128, 32] -> [32, 128] -> HBM ----
            at_ps = sm_psum.tile([NCOL, 128], aldt, tag="sm")
            nc.tensor.matmul(at_ps[:], alpha[:],
                             identity_r[:] if aldt == f32r else identity[:],
                             is_transpose=True)
            al_sb = outb.tile([NCOL, 128], f32, tag="alsb")
            nc.vector.tensor_copy(out=al_sb[:], in_=at_ps[:])
            nc.sync.dma_start(out=alpha_o[b].rearrange("(r c) -> r c", c=128),
                              in_=al_sb[:])

        for b in range(BPC):
            phase1(b)
            if b > 0:
                phase23(b - 1)
        phase23(BPC - 1)

    nc.compile()
    return nc


def _get_nc():
    key = (PREC, AWE)
    if key not in _NC_CACHE:
        _NC_CACHE[key] = build_nc(*key)
    return _NC_CACHE[key]


def make_in_maps(encoder_out, mask, decoder_hidden, W_enc, b_enc, W_dec,
                 b_dec, w_full, b_full, mask_need):
    """Shard the full inputs into per-core input maps."""
    enc = np.ascontiguousarray(np.asarray(encoder_out, dtype=np.float32))
    mask = np.asarray(mask, dtype=np.float32)
    dh = np.asarray(decoder_hidden, dtype=np.float32)
    mask_need = int(np.asarray(mask_need))
    if mask_need != 1:
        mask = np.ones_like(mask)
    shared = {
        "wenc": np.ascontiguousarray(np.asarray(W_enc, dtype=np.float32)),
        "benc": np.ascontiguousarray(np.asarray(b_enc, dtype=np.float32)),
        "wdec": np.ascontiguousarray(np.asarray(W_dec, dtype=np.float32)),
        "bdec": np.ascontiguousarray(np.asarray(b_dec, dtype=np.float32)),
        "wful": np.ascontiguousarray(np.asarray(w_full, dtype=np.float32)),
        "bful": np.asarray(b_full, dtype=np.float32).reshape(1),
    }
    in_maps = []
    for i in range(NCORES):
        sl = slice(i * BPC, (i + 1) * BPC)
        in_maps.append({
            "enc": np.ascontiguousarray(enc[sl]),
            "mask": np.ascontiguousarray(mask[sl]),
            "dh": np.ascontiguousarray(dh[sl]),
            **shared,
        })
    return in_maps


def kernel(**inputs):
    global LAST_RESULT
    _ensure_path()
    from concourse.bass_utils import run_bass_kernel_spmd

    nc = _get_nc()
    in_maps = make_in_maps(**inputs)
    res = run_bass_kernel_spmd(nc, in_maps, list(range(NCORES)))
    LAST_RESULT = res
    awe = np.concatenate([np.asarray(r["awe_out"]) for r in res.results], axis=0)
    alpha = np.concatenate([np.asarray(r["alpha_out"]) for r in res.results],
                           axis=0)
    return awe.astype(np.float32), alpha.astype(np.float32)
